# revision 13
# baseline (speedup 1.0000x reference)
"""AdjustableConvolution2d Trainium2 kernel.

Data-parallel over batch: 8 samples -> 8 NeuronCores, no collectives.

Per-core pipeline (one sample, c=256 channels, 64x64 spatial):
  1. filter logits in one fused matmul: host constant-folds
     W_comb=(Wt@Wf)/100, b_comb=(bt@Wf+bf)/100, device computes
     temp @ W_comb + b_comb (bias via a K=1 accumulate row), then
     softmax over the 9 taps laid out as per-partition scalars.
  2. depthwise 3x3 with per-(sample,channel) taps: computed on the
     TensorEngine as diag(filt[:,tap]) @ shifted_view(padded_image) in bf16,
     9 taps accumulated in fp32 PSUM.
  3. 1x1 channel combine: WcT chunks as bf16 stationary operand, accumulate
     over channel chunks in PSUM, add bias on ACT/DVE while copying to SBUF.
Host-side prep: layout, bf16 rounding of matmul operands, and constant
folding of the static weight-weight product.
"""

import numpy as np
import ml_dtypes

BS, C, H, W = 8, 256, 64, 64
KK = 3
P = 128
CC = C // P            # channel chunks of 128
HP, WP = H + 2, W + 2  # zero-padded spatial
SQ, TIN = 32, 256
CKK = C * KK * KK      # 2304
RS = 8                 # output rows per hw-slice
NS = RS * W            # 512 elements per hw-slice
NSL = H // RS          # 8 slices
GRP = 4                # hw-slices per psum group (stationary-weight reuse)

# blob_a column layout (fp32 columns, 128 partitions)
A_WCT0, A_WCT1 = 0, 256        # Wc.T as bf16 pairs packed in fp32 words
A_BC0, A_BC1 = 256, 258        # bc [p, cc]
A_TMP = 258                    # temp_feat bf16 pair [p, cc]
A_N = 259

_CACHE = {}


def _build():
    from contextlib import ExitStack

    import concourse.bass as bass
    import concourse.bacc as bacc
    import concourse.mybir as mybir
    import concourse.tile as tile
    from concourse import masks

    dt = mybir.dt
    f32 = dt.float32
    bf16 = dt.bfloat16
    AF = mybir.ActivationFunctionType
    ALU = mybir.AluOpType
    AX = mybir.AxisListType

    nc = bacc.Bacc(
        "TRN2", target_bir_lowering=False, debug=False, enable_asserts=False
    )

    NF = 512
    NCH = CKK // NF + (1 if CKK % NF else 0)  # 5 chunks of <=512 logits
    img_d = nc.dram_tensor("img", [C, HP * WP], bf16, kind="ExternalInput")
    bla_d = nc.dram_tensor("bla", [P, A_N], f32, kind="ExternalInput")
    wcb_d = nc.dram_tensor("wcb", [NCH, C, NF], bf16, kind="ExternalInput")
    bcb_d = nc.dram_tensor("bcb", [1, NCH * NF], f32, kind="ExternalInput")
    out_d = nc.dram_tensor("out", [C, H, W], f32, kind="ExternalOutput")

    with tile.TileContext(nc) as tc, ExitStack() as ctx:
        constp = ctx.enter_context(tc.tile_pool(name="const", bufs=1))
        imgp = ctx.enter_context(tc.tile_pool(name="img", bufs=1))
        filtp = ctx.enter_context(tc.tile_pool(name="filt", bufs=1))
        sps = ctx.enter_context(
            tc.tile_pool(name="spsum", bufs=2, space=bass.MemorySpace.PSUM)
        )
        midps = ctx.enter_context(
            tc.tile_pool(name="midps", bufs=4, space=bass.MemorySpace.PSUM)
        )
        outps = ctx.enter_context(
            tc.tile_pool(name="outps", bufs=2, space=bass.MemorySpace.PSUM)
        )
        midsb = ctx.enter_context(tc.tile_pool(name="midsb", bufs=16))
        outsb = ctx.enter_context(tc.tile_pool(name="outsb", bufs=3))

        # ---- weights first on the scalar-engine DMA queue, then image ------
        bla = constp.tile([P, A_N], f32)
        nc.scalar.dma_start(bla[:], bla_d[:, :])
        bcb = constp.tile([1, NCH * NF], f32)  # b_comb row
        nc.scalar.dma_start(bcb[:], bcb_d[:, :])
        wcb = constp.tile([P, NCH, CC, NF], bf16)  # W_comb, chunked
        for j in range(NCH):
            nc.scalar.dma_start(
                wcb[:, j], wcb_d[j].rearrange("(cc p) n -> p cc n", p=P)
            )

        wct_sb = bla[:, A_WCT0:A_WCT1].bitcast(bf16)  # [128, 512]
        wct_v = wct_sb.rearrange("p (cc o) -> p cc o", cc=CC)
        bc_v = bla[:, A_BC0:A_BC1]
        temp_v = bla[:, A_TMP : A_TMP + 1].bitcast(bf16)  # [128, 2] bf16

        ident = constp.tile([P, P], f32)
        masks.make_identity(nc, ident[:])

        # image on the sync queue; weights go concurrently on scalar
        img_sb = imgp.tile([P, CC, HP * WP], bf16)
        imgv = []
        for cc in range(CC):
            v = img_sb[:, cc, :].rearrange("p (r w) -> p r w", w=WP)
            imgv.append(v)
            nc.sync.dma_start(img_sb[:, cc, :], img_d[cc * P : (cc + 1) * P, :])

        # ---- filter logits: fused matmul chain, bias folded in the copy ----
        flt_sb = filtp.tile([1, CKK], f32)
        for j in range(NCH):
            c0 = j * NF
            n = min(NF, CKK - c0)
            f_ps = sps.tile([1, NF], f32, name="fps", tag="small")
            for cc in range(CC):
                nc.tensor.matmul(
                    f_ps[:, :n],
                    temp_v[:, cc : cc + 1],
                    wcb[:, j, cc, :n],
                    start=(cc == 0),
                    stop=(cc == CC - 1),
                )
            nc.vector.scalar_tensor_tensor(
                flt_sb[:, c0 : c0 + n],
                f_ps[:, :n],
                1.0,
                bcb[:, c0 : c0 + n],
                op0=ALU.mult,
                op1=ALU.add,
            )

        # per-chunk pipeline: scatter -> softmax -> diag, cc0 first so the
        # TensorEngine starts as early as possible
        fT = filtp.tile([P, CC, KK * KK], f32)
        e = filtp.tile([P, CC, KK * KK], f32)
        s = filtp.tile([P, CC], f32)
        r = filtp.tile([P, CC], f32)
        filtn = filtp.tile([P, CC, KK * KK], f32)
        diag = constp.tile([P, CC, KK * KK, P], bf16)
        for cc in range(CC):
            nc.sync.dma_start(
                fT[:, cc, :],
                flt_sb[:, cc * P * KK * KK : (cc + 1) * P * KK * KK].rearrange(
                    "one (p k) -> one p k", k=KK * KK
                ),
            )
            nc.scalar.activation(e[:, cc], fT[:, cc, :], AF.Exp)
            nc.vector.reduce_sum(s[:, cc : cc + 1], e[:, cc], axis=AX.X)
            nc.vector.reciprocal(r[:, cc : cc + 1], s[:, cc : cc + 1])
            nc.vector.tensor_scalar_mul(
                filtn[:, cc], e[:, cc], r[:, cc : cc + 1]
            )
            splits = ((0, 3), (3, 9)) if cc == 0 else ((0, 9),)
            for lo, hi in splits:
                nc.vector.tensor_tensor(
                    diag[:, cc, lo:hi],
                    ident[:, :].unsqueeze(1).to_broadcast((P, hi - lo, P)),
                    filtn[:, cc, lo:hi].unsqueeze(2).to_broadcast(
                        (P, hi - lo, P)
                    ),
                    op=ALU.mult,
                )

        # ---- main loop ------------------------------------------------------
        out_flat = out_d[:, :, :].rearrange("c h w -> c (h w)")
        for g in range(NSL // GRP):
            mids = [[None] * GRP for _ in range(CC)]
            for cc in range(CC):
                mts = [
                    midps.tile([P, NS], f32, name="mid", tag="mid")
                    for _ in range(GRP)
                ]
                for t9 in range(KK * KK):
                    di, dj = t9 // KK, t9 % KK
                    lhsT = diag[:, cc, t9, :]
                    for s4 in range(GRP):
                        hs = g * GRP + s4
                        r0 = RS * hs + di
                        rhs = imgv[cc][:, r0 : r0 + RS, dj : dj + W]
                        nc.tensor.matmul(
                            mts[s4][:],
                            lhsT,
                            rhs,
                            start=(t9 == 0),
                            stop=(t9 == KK * KK - 1),
                        )
                for s4 in range(GRP):
                    m = midsb.tile([P, NS], bf16, name="midt", tag="midt")
                    if s4 % 2 == 0:
                        nc.vector.tensor_copy(m[:], mts[s4][:])
                    else:
                        nc.scalar.copy(m[:], mts[s4][:])
                    mids[cc][s4] = m
            for oc in range(CC):
                obg = outsb.tile([P, GRP * NS], f32, name="obg", tag="obg")
                for s4 in range(GRP):
                    o_ps = outps.tile([P, NS], f32, name="ops", tag="ops")
                    for cc in range(CC):
                        nc.tensor.matmul(
                            o_ps[:],
                            wct_v[:, cc, oc * P : (oc + 1) * P],
                            mids[cc][s4][:],
                            start=(cc == 0),
                            stop=(cc == CC - 1),
                        )
                    dst = obg[:, s4 * NS : (s4 + 1) * NS]
                    if oc == 0:
                        nc.scalar.activation(
                            dst, o_ps[:], AF.Identity, bias=bc_v[:, oc : oc + 1]
                        )
                    else:
                        nc.vector.tensor_scalar_add(
                            dst, o_ps[:], bc_v[:, oc : oc + 1]
                        )
                for hh in range(GRP):
                    eng = nc.sync if (oc + hh) % 2 == 0 else nc.scalar
                    eng.dma_start(
                        out_flat[
                            oc * P : (oc + 1) * P,
                            g * GRP * NS + hh * NS : g * GRP * NS
                            + (hh + 1) * NS,
                        ],
                        obg[:, hh * NS : (hh + 1) * NS],
                    )

    nc.compile()
    return nc


def _get_nc():
    if "nc" not in _CACHE:
        _CACHE["nc"] = _build()
    return _CACHE["nc"]


def _prep_in_maps(image_feat, temp_feat, Wt, bt, Wf, bf, Wc, bc):
    f = lambda a: np.ascontiguousarray(np.asarray(a, dtype=np.float32))
    image_feat = f(image_feat)
    temp_feat = f(temp_feat)

    img_pad = np.zeros((BS, C, HP, WP), ml_dtypes.bfloat16)
    img_pad[:, :, 1 : H + 1, 1 : W + 1] = image_feat.astype(ml_dtypes.bfloat16)
    img_pad = img_pad.reshape(BS, C, HP * WP)

    # constant-fold the static weight-weight product (weights are module
    # constants; the per-sample compute stays on device)
    NF = 512
    NCH = CKK // NF + (1 if CKK % NF else 0)
    w_comb = ((f(Wt) @ f(Wf)) / 100.0).astype(ml_dtypes.bfloat16)  # [256, 2304]
    w_comb_ch = np.zeros((NCH, C, NF), ml_dtypes.bfloat16)
    for j in range(NCH):
        n = min(NF, CKK - j * NF)
        w_comb_ch[j, :, :n] = w_comb[:, j * NF : j * NF + n]
    b_comb_full = np.zeros((1, NCH * NF), np.float32)
    b_comb_full[0, :CKK] = (f(bt) @ f(Wf) + f(bf)) / 100.0

    blob_a = np.zeros((P, A_N), np.float32)
    wct = np.ascontiguousarray(f(Wc).T).astype(ml_dtypes.bfloat16)  # [c, o]
    wct_p = wct.reshape(CC, P, C).transpose(1, 0, 2).reshape(P, CC * C)
    blob_a[:, A_WCT0:A_WCT1] = np.ascontiguousarray(wct_p).view(np.float32)
    blob_a[:, A_BC0:A_BC1] = f(bc).reshape(CC, P).T

    in_maps = []
    for i in range(BS):
        ba = blob_a.copy()
        tb = (
            temp_feat[i]
            .reshape(CC, P)
            .T.astype(ml_dtypes.bfloat16)
        )  # [128, 2] bf16
        ba[:, A_TMP] = np.ascontiguousarray(tb).view(np.float32)[:, 0]
        in_maps.append(
            {"img": img_pad[i], "bla": ba, "wcb": w_comb_ch, "bcb": b_comb_full}
        )
    return in_maps


def kernel(image_feat, temp_feat, Wt, bt, Wf, bf, Wc, bc):
    from concourse.bass_utils import run_bass_kernel_spmd

    nc = _get_nc()
    in_maps = _prep_in_maps(image_feat, temp_feat, Wt, bt, Wf, bf, Wc, bc)
    res = run_bass_kernel_spmd(nc, in_maps, core_ids=list(range(BS)))
    _CACHE["last_result"] = res
    out = np.stack([res.results[i]["out"] for i in range(BS)], axis=0)
    return out.astype(np.float32)
